# revision 7
# baseline (speedup 1.0000x reference)
"""Jagged log-softmax over 65536 segments of a flat 2**25 logits array.

Strategy
--------
Segment boundaries (prefix_sum) are known on the host at call time, so the
Bass program is specialized to them:

* Sort segments by length; pack 128 segments per tile (one segment per SBUF
  partition row).  512 tiles -> 8 cores x 64 slots, tile t -> core t%8,
  slot t//8, so all cores share one program (one NEFF) with identical
  compile-time slot widths.
* Slot width C_s = max segment length among the 1024 sorted segments in that
  slot (sorted order => ~0.8% padding).  Rows are padded with -100.0 so
  exp(pad) == 0 and the padded columns never contribute to the row sum.
* Device per group of 8 slots: one big DMA in ([128, ~4..6K] f32, ~2MB),
  ScalarE Exp with accum_out (fused exp + row-sum), ScalarE Ln, DVE
  tensor_scalar subtract (out = x - logZ row-broadcast), one big DMA out.
  log-softmax without max-subtraction is exact for N(0,1) logits (no
  overflow possible), matching the reference to fp32 rounding.
* Host scatters the unpadded columns back into the flat output.
"""

import os
from contextlib import ExitStack

import numpy as np

N_TOTAL = 33554432
NSEG = 65536
NCORES = 8
ROWS = 128
TILES = NSEG // ROWS            # 512
SLOTS = TILES // NCORES         # 64 slots per core
GROUP = 8                       # slots per DMA group
NGROUPS = SLOTS // GROUP        # 8 groups per core
PAD_VAL = np.float32(-100.0)

LAST_RESULT = None              # BassKernelResults of the most recent run
LAST_RUN_S = None               # wall seconds of the most recent device run


def _build_bass(slot_widths, W_total):
    import concourse.bacc as bacc
    import concourse.mybir as mybir
    import concourse.tile as tile

    f32 = mybir.dt.float32
    Exp = mybir.ActivationFunctionType.Exp
    Ln = mybir.ActivationFunctionType.Ln

    off = np.zeros(SLOTS + 1, np.int64)
    off[1:] = np.cumsum(slot_widths)

    nc = bacc.Bacc("TRN2", target_bir_lowering=False)
    xin = nc.dram_tensor("xin", [ROWS, W_total], f32, kind="ExternalInput")
    yout = nc.dram_tensor("yout", [ROWS, W_total], f32, kind="ExternalOutput")

    repeat = int(os.environ.get("KERNEL_REPEAT", "1"))

    with ExitStack() as ctx:
        tc = ctx.enter_context(tile.TileContext(nc))
        xpool = ctx.enter_context(tc.tile_pool(name="xpool", bufs=3))
        epool = ctx.enter_context(tc.tile_pool(name="epool", bufs=2))
        spool = ctx.enter_context(tc.tile_pool(name="spool", bufs=4))

        if repeat > 1:
            ctx.enter_context(tc.For_i(0, repeat, 1))

        for q in range(NGROUPS):
            s0 = q * GROUP
            goff = int(off[s0])
            gw = int(off[s0 + GROUP] - goff)

            xt = xpool.tile([ROWS, gw], f32, tag="xt", name=f"xt{q}")
            nc.sync.dma_start(xt[:], xin[:, goff:goff + gw])

            et = epool.tile([ROWS, gw], f32, tag="et", name=f"et{q}")
            sums = spool.tile([ROWS, GROUP], f32, tag="sums", name=f"sums{q}")
            for g in range(GROUP):
                a = int(off[s0 + g] - goff)
                L = int(slot_widths[s0 + g])
                nc.scalar.activation(
                    et[:, a:a + L], xt[:, a:a + L], Exp,
                    accum_out=sums[:, g:g + 1],
                )

            logz = spool.tile([ROWS, GROUP], f32, tag="logz", name=f"logz{q}")
            nc.scalar.activation(logz[:], sums[:], Ln)

            for g in range(GROUP):
                a = int(off[s0 + g] - goff)
                L = int(slot_widths[s0 + g])
                # tensor_tensor with a stride-0 broadcast of logz: the
                # TensorScalarPtr form hits a walrus "too many sync waits"
                # codegen limit, plain TT does not.
                nc.vector.tensor_sub(
                    xt[:, a:a + L], xt[:, a:a + L],
                    logz[:, g:g + 1].broadcast_to([ROWS, L]),
                )

            nc.sync.dma_start(yout[:, goff:goff + gw], xt[:])

    if not nc.is_finalized():
        nc.finalize()
    return nc


def kernel(logits, prefix_sum):
    global LAST_RESULT
    from concourse.bass_utils import run_bass_kernel_spmd

    x = np.ascontiguousarray(np.asarray(logits, dtype=np.float32).reshape(-1))
    prefix = np.asarray(prefix_sum).astype(np.int64).reshape(-1)
    assert x.shape[0] == N_TOTAL and prefix.shape[0] == NSEG

    starts = np.empty(NSEG, np.int64)
    starts[0] = 0
    starts[1:] = prefix[:-1]
    lens = prefix - starts

    order = np.argsort(lens, kind="stable")
    lens_sorted = lens[order]
    slot_widths = lens_sorted.reshape(SLOTS, ROWS * NCORES).max(axis=1)
    W_total = int(slot_widths.sum())
    off = np.zeros(SLOTS + 1, np.int64)
    off[1:] = np.cumsum(slot_widths)

    x_ext = np.concatenate([x, np.asarray([PAD_VAL], np.float32)])

    # Pack: slot s holds sorted positions [1024s, 1024(s+1)); core c gets the
    # contiguous 128 positions starting at 1024s + 128c.
    bufs = np.empty((NCORES, ROWS, W_total), np.float32)
    for s in range(SLOTS):
        C = int(slot_widths[s])
        segs = order[1024 * s: 1024 * (s + 1)].reshape(NCORES, ROWS)
        cols = np.arange(C, dtype=np.int64)
        idx = starts[segs][:, :, None] + cols[None, None, :]
        mask = cols[None, None, :] < lens[segs][:, :, None]
        np.copyto(idx, N_TOTAL, where=~mask)
        bufs[:, :, off[s]:off[s] + C] = x_ext[idx]

    nc = _build_bass(slot_widths, W_total)
    in_maps = [{"xin": bufs[c]} for c in range(NCORES)]
    import time as _time
    global LAST_RUN_S
    _t0 = _time.perf_counter()
    LAST_RESULT = run_bass_kernel_spmd(
        nc, in_maps, core_ids=list(range(NCORES)),
        trace=bool(int(os.environ.get("KERNEL_TRACE", "0"))),
    )
    LAST_RUN_S = _time.perf_counter() - _t0
    results = LAST_RESULT.results

    out = np.empty(N_TOTAL, np.float32)
    for s in range(SLOTS):
        C = int(slot_widths[s])
        segs = order[1024 * s: 1024 * (s + 1)].reshape(NCORES, ROWS)
        cols = np.arange(C, dtype=np.int64)
        idx = starts[segs][:, :, None] + cols[None, None, :]
        mask = cols[None, None, :] < lens[segs][:, :, None]
        y = np.stack([results[c]["yout"][:, off[s]:off[s] + C]
                      for c in range(NCORES)])
        out[idx[mask]] = y[mask]
    return out
